# revision 18
# baseline (speedup 1.0000x reference)
# Multi-head attention (RoPE, causal) Trainium2 Bass kernel.
#
# Sharding: 8 cores = 4 batches x 2 head-groups (8 heads each).
# Core c: batch c//2, heads (c%2)*8 .. +8. Each core computes a partial
# output projection (Wo row-parallel); host sums core pairs.
#
# Per-core dataflow (matmuls in float32r => ~1 cycle/row at free-dim>=256):
#   1. x^T via PE transposes (f32r), resident in SBUF [128, 16, 2048]
#   2. Q^T/K^T = Wq^T x^T per head (+fused RoPE via signed-permutation
#      matmul + DVE combine), V natural-layout per head-pair; spill to DRAM
#   3. attention in S^T=[k,q] layout: S^T tile = matmul(lhsT=K^T slice,
#      rhs=Q^T block); exp on ACT (ACT is exp-only to avoid activation-table
#      reloads); causal mask (0/1 multiply) only on diagonal tiles; PV
#      accumulates O^T = matmul(lhsT=V tile, rhs=P^T); softmax denominators
#      via all-ones matmul (sums broadcast across partitions for free);
#      divide O^T blocks by sums.
#   4. partial out = matmul(lhsT=O^T tiles, rhs=Wo) accumulated over heads.
# All PSUM->SBUF copies are on DVE. Matmul emission order favors
# consecutive identical moving operands (HW streams them cheaper).
import math

import numpy as np

P = 128
L = 2048
D = 2048
NH = 8  # heads per core
NK = D // P  # 16 contraction subtiles
NL = L // P  # 16 L chunks
NQ = 4  # q blocks
QB = 512  # q block size
NPAIR = 4

_CACHE = {}


def _host_consts():
    i = np.arange(64, dtype=np.float32)
    inv_freq = (1.0 / (10000.0 ** (2.0 * i / 128.0))).astype(np.float32)
    t = np.arange(L, dtype=np.float32)
    freqs = np.outer(t, inv_freq)  # [L, 64]
    cos = np.cos(freqs).astype(np.float32)
    sin = np.sin(freqs).astype(np.float32)
    cost = np.ascontiguousarray(np.concatenate([cos, cos], axis=1).T)  # [128, L]
    sint = np.ascontiguousarray(np.concatenate([sin, sin], axis=1).T)
    rot = np.zeros((P, P), np.float32)
    rot[np.arange(64) + 64, np.arange(64)] = -1.0
    rot[np.arange(64), np.arange(64) + 64] = 1.0
    ones = np.ones((P, P), np.float32)
    masks = np.zeros((4, P, QB), np.float32)
    kk = np.arange(P)[:, None]
    qq = np.arange(QB)[None, :]
    for tt in range(4):
        masks[tt] = (P * tt + kk <= qq).astype(np.float32)
    return cost, sint, rot, ones, masks


def _build_module(repeats=1):
    import concourse.bacc as bacc
    import concourse.tile as tile
    import concourse.mybir as mybir
    from concourse.masks import make_identity

    f32 = mybir.dt.float32
    f32r = mybir.dt.float32r
    Exp = mybir.ActivationFunctionType.Exp

    nc = bacc.Bacc("TRN2", target_bir_lowering=False, debug=False,
                   enable_asserts=False, num_devices=8)

    x_t = nc.dram_tensor("x", [L, D], f32r, kind="ExternalInput").ap()
    wq_t = nc.dram_tensor("wq", [D, NH * P], f32r, kind="ExternalInput").ap()
    wk_t = nc.dram_tensor("wk", [D, NH * P], f32r, kind="ExternalInput").ap()
    wv_t = nc.dram_tensor("wv", [D, NH * P], f32r, kind="ExternalInput").ap()
    wo_t = nc.dram_tensor("wo", [NH * P, D], f32r, kind="ExternalInput").ap()
    cost_t = nc.dram_tensor("cost", [P, L], f32, kind="ExternalInput").ap()
    sint_t = nc.dram_tensor("sint", [P, L], f32, kind="ExternalInput").ap()
    rot_t = nc.dram_tensor("rot", [P, P], f32r, kind="ExternalInput").ap()
    ones_t = nc.dram_tensor("ones", [P, P], f32r, kind="ExternalInput").ap()
    masks_t = nc.dram_tensor("masks", [4, P, QB], f32, kind="ExternalInput").ap()
    out_t = nc.dram_tensor("out", [L, D], f32, kind="ExternalOutput").ap()

    with tile.TileContext(nc) as tc:
        with tc.tile_pool(name="const", bufs=1) as const, \
             tc.tile_pool(name="dram", bufs=1, space="DRAM") as dram:
            ident = const.tile([P, P], f32)
            make_identity(nc, ident)
            cost = const.tile([P, L], f32)
            nc.gpsimd.dma_start(cost[:], cost_t)
            sint = const.tile([P, L], f32)
            nc.gpsimd.dma_start(sint[:], sint_t)
            rot = const.tile([P, P], f32r)
            nc.gpsimd.dma_start(rot[:], rot_t)
            ones = const.tile([P, P], f32r)
            nc.gpsimd.dma_start(ones[:], ones_t)
            maskt = const.tile([P, 4, QB], f32)
            nc.gpsimd.dma_start(maskt[:], masks_t.rearrange("t p q -> p t q"))

            qT_d = dram.tile([NH, P, L], f32r)
            kT_d = dram.tile([NH, P, L], f32r)
            v_d = dram.tile([NPAIR, P, NL, 256], f32r)

            for _rep in range(repeats):
                # ---- phases 1+2: x^T resident ----
                with tc.tile_pool(name="xtp", bufs=1) as xtp:
                    xt = xtp.tile([P, NK, L], f32r)

                    # phase 1: x^T via PE transposes
                    xr = x_t.rearrange("(lo p) d -> p lo d", p=P)
                    with tc.tile_pool(name="p1", bufs=3) as p1, \
                         tc.tile_pool(name="ps1", bufs=6, space="PSUM") as ps1:
                        for lc in range(NL):
                            xin = p1.tile([P, D], f32r, tag="xin")
                            nc.sync.dma_start(xin[:], xr[:, lc, :])
                            for dt_ in range(NK):
                                ps = ps1.tile([P, P], f32, tag="tp")
                                nc.tensor.transpose(
                                    ps[:],
                                    xin.bitcast(f32)[:, dt_ * P:(dt_ + 1) * P],
                                    ident[:])
                                dst = xt[:, dt_, lc * P:(lc + 1) * P]
                                if (lc + dt_) % 2 == 0:
                                    nc.vector.tensor_copy(dst, ps[:])
                                else:
                                    nc.scalar.copy(dst, ps[:])

                    # phase 2: per pair: Q^T/K^T (+RoPE) for both heads,
                    # then V for the pair (k-streamed Wv, 4-long same-rhs runs)
                    wvr = wv_t.rearrange("(ko p) m -> p ko m", p=P)
                    wqr = wq_t.rearrange("(ko p) m -> p ko m", p=P)
                    wkr = wk_t.rearrange("(ko p) m -> p ko m", p=P)
                    with tc.tile_pool(name="p2a", bufs=1) as p2a, \
                         tc.tile_pool(name="ps2a", bufs=1, space="PSUM") as ps2a, \
                         tc.tile_pool(name="p2b", bufs=2) as p2b, \
                         tc.tile_pool(name="ps2b", bufs=2, space="PSUM") as ps2b:
                        for pr in range(NPAIR):
                            for wr, dst_d in ((wqr, qT_d), (wkr, kT_d)):
                                h0, h1 = 2 * pr, 2 * pr + 1
                                wts = []
                                for hh in (h0, h1):
                                    wt = p2b.tile([P, NK, P], f32r,
                                                  tag=f"w{hh % 2}",
                                                  name=f"wt_{hh % 2}", bufs=1)
                                    nc.sync.dma_start(
                                        wt[:], wr[:, :, hh * P:(hh + 1) * P])
                                    wts.append(wt)
                                for n in range(NQ):
                                    ns = slice(n * QB, (n + 1) * QB)
                                    pss = []
                                    for d_ in range(2):
                                        pp = ps2b.tile([P, QB], f32,
                                                       tag=f"proj{d_}",
                                                       name=f"proj_{d_}",
                                                       bufs=2)
                                        pss.append(pp)
                                    # both heads' matmuls share rhs per k
                                    for k in range(NK):
                                        for d_ in range(2):
                                            nc.tensor.matmul(
                                                pss[d_][:],
                                                lhsT=wts[d_][:, k, :],
                                                rhs=xt[:, k, ns],
                                                start=(k == 0),
                                                stop=(k == NK - 1))
                                    for d_ in range(2):
                                        h = (h0, h1)[d_]
                                        ps = pss[d_]
                                        raw = p2b.tile([P, QB], f32r,
                                                       tag="raw", bufs=1)
                                        nc.vector.tensor_copy(raw[:], ps[:])
                                        pm = ps2b.tile([P, QB], f32,
                                                       tag=f"proj{d_}",
                                                       name=f"pm_{d_}",
                                                       bufs=2)
                                        nc.tensor.matmul(pm[:], lhsT=rot[:],
                                                         rhs=raw[:],
                                                         start=True, stop=True)
                                        t1 = p2b.tile([P, QB], f32, tag="t1",
                                                      bufs=1)
                                        nc.vector.tensor_mul(
                                            t1[:], raw.bitcast(f32)[:],
                                            cost[:, ns])
                                        t2 = p2b.tile([P, QB], f32, tag="t2",
                                                      bufs=1)
                                        nc.vector.tensor_mul(t2[:], pm[:],
                                                             sint[:, ns])
                                        ro = p2b.tile([P, QB], f32r,
                                                      tag="ro", bufs=3)
                                        nc.vector.tensor_add(ro[:],
                                                             t1[:], t2[:])
                                        nc.sync.dma_start(dst_d[h, :, ns],
                                                          ro[:])
                            # V for this pair
                            vsb = p2a.tile([P, NL, 256], f32r, tag="vsb",
                                           bufs=1)
                            for lg in range(4):
                                pss = []
                                for l4 in range(4):
                                    pp = ps2a.tile([P, 256], f32,
                                                   tag=f"vps{l4}",
                                                   bufs=1, name=f"vps_{l4}")
                                    pss.append(pp)
                                for k in range(NK):
                                    wvk = p2a.tile([P, 256], f32r, tag="wvk",
                                                   bufs=4)
                                    nc.sync.dma_start(
                                        wvk[:],
                                        wvr[:, k, pr * 256:(pr + 1) * 256])
                                    for l4 in range(4):
                                        lc = lg * 4 + l4
                                        nc.tensor.matmul(
                                            pss[l4][:],
                                            lhsT=xt[:, k, lc * P:(lc + 1) * P],
                                            rhs=wvk[:],
                                            start=(k == 0), stop=(k == NK - 1))
                                for l4 in range(4):
                                    lc = lg * 4 + l4
                                    nc.vector.tensor_copy(
                                        vsb[:, lc, :], pss[l4][:])
                            nc.sync.dma_start(v_d[pr], vsb[:])

                # ---- phases 3+4: O^T stays resident in SBUF ----
                with tc.tile_pool(name="otp", bufs=1) as otp:
                  oTs = []
                  for h in range(NH):
                      o_ = otp.tile([P, L], f32r, tag=f"oT{h}", name=f"oT_{h}")
                      oTs.append(o_)
                  # ---- phase 3: attention, S^T layout ----
                  with tc.tile_pool(name="p3", bufs=2) as p3, \
                       tc.tile_pool(name="p3t", bufs=4) as p3t, \
                       tc.tile_pool(name="ps3", bufs=2, space="PSUM") as ps3:
                    for h in range(NH):
                        oTh = oTs[h]
                        pr, hp = divmod(h, 2)
                        qTh = p3.tile([P, L], f32r, tag="qTh", bufs=2)
                        nc.sync.dma_start(qTh[:], qT_d[h])
                        kTh = p3.tile([P, L], f32r, tag="kTh", bufs=2)
                        nc.sync.dma_start(kTh[:], kT_d[h])
                        vh = p3.tile([P, NL, P], f32r, tag="vh", bufs=2)
                        nc.sync.dma_start(vh[:],
                                          v_d[pr, :, :, hp * P:(hp + 1) * P])
                        for j in range(NQ):
                            qs = slice(j * QB, (j + 1) * QB)
                            ops_t = ps3.tile([P, QB], f32, tag="ops", bufs=2)
                            sums_t = ps3.tile([P, QB], f32, tag="sums", bufs=2)
                            nt = 4 * j + 4
                            # groups of 4 k-tiles: S matmuls adjacent so
                            # the identical rhs (qTh block) streams cheaply
                            for i0 in range(0, nt, 4):
                                sps_g = []
                                pts = []
                                for d_ in range(4):
                                    i = i0 + d_
                                    sp = ps3.tile([P, QB], f32, tag=f"sps{d_}",
                                                  bufs=1, name=f"sp_{d_}")
                                    nc.tensor.matmul(
                                        sp[:],
                                        lhsT=kTh[:, i * P:(i + 1) * P],
                                        rhs=qTh[:, qs],
                                        start=True, stop=True)
                                    sps_g.append(sp)
                                for d_ in range(4):
                                    i = i0 + d_
                                    pt = p3t.tile([P, QB], f32r, tag="pt",
                                                  bufs=6)
                                    nc.scalar.activation(pt[:],
                                                         sps_g[d_][:], Exp)
                                    if i >= 4 * j:
                                        tt = i - 4 * j
                                        w = P * (tt + 1)
                                        nc.vector.tensor_mul(
                                            pt[:, :w],
                                            pt.bitcast(f32)[:, :w],
                                            maskt[:, tt, :w])
                                    pts.append(pt)
                                for d_ in range(4):
                                    i = i0 + d_
                                    nc.tensor.matmul(
                                        ops_t[:], lhsT=vh[:, i, :],
                                        rhs=pts[d_][:],
                                        start=(i == 0), stop=(i == nt - 1))
                                    nc.tensor.matmul(
                                        sums_t[:], lhsT=ones[:],
                                        rhs=pts[d_][:],
                                        start=(i == 0), stop=(i == nt - 1))
                            rec = p3t.tile([P, QB], f32, tag="rec", bufs=2)
                            nc.vector.reciprocal(rec[:], sums_t[:])
                            nc.vector.tensor_mul(oTh[:, qs], ops_t[:], rec[:])

                  # ---- phase 4: output projection ----
                  wor = wo_t.rearrange("(h p) n -> p h n", p=P)
                  outr = out_t.rearrange("(lo p) n -> p lo n", p=P)
                  with tc.tile_pool(name="p4", bufs=1) as p4, \
                       tc.tile_pool(name="ps4", bufs=1, space="PSUM") as ps4:
                    wo_sb = p4.tile([P, NH, D], f32r, tag="wo")
                    for h in range(NH):
                        nc.sync.dma_start(wo_sb[:, h, :], wor[:, h, :])
                    ots = oTs
                    for lg in range(4):
                        for n in range(NQ):
                            ns = slice(n * QB, (n + 1) * QB)
                            pss = []
                            for l4 in range(4):
                                pp = ps4.tile([P, QB], f32, tag=f"ops4_{l4}",
                                              bufs=2, name=f"op4_{l4}")
                                pss.append(pp)
                            # h accumulates; inner l4 shares rhs (wo slice)
                            for h in range(NH):
                                for l4 in range(4):
                                    lc = lg * 4 + l4
                                    nc.tensor.matmul(
                                        pss[l4][:],
                                        lhsT=ots[h][:, lc * P:(lc + 1) * P],
                                        rhs=wo_sb[:, h, ns],
                                        start=(h == 0), stop=(h == NH - 1))
                            for l4 in range(4):
                                lc = lg * 4 + l4
                                osb = p4.tile([P, QB], f32, tag="osb", bufs=4)
                                nc.vector.tensor_copy(osb[:], pss[l4][:])
                                nc.sync.dma_start(outr[:, lc, ns], osb[:])

    nc.compile()
    return nc


def get_module(repeats=1):
    key = ("nc", repeats)
    if key not in _CACHE:
        _CACHE[key] = _build_module(repeats)
    return _CACHE[key]


def make_in_maps(x, Wq, Wk, Wv, Wo):
    cost, sint, rot, ones, masks = _host_consts()
    s = np.float32(1.0 / math.sqrt(128.0))
    in_maps = []
    for c in range(8):
        b, hg = divmod(c, 2)
        cs = slice(hg * 1024, (hg + 1) * 1024)
        in_maps.append({
            "x": np.ascontiguousarray(x[b], np.float32),
            "wq": np.ascontiguousarray(Wq[:, cs] * s, np.float32),
            "wk": np.ascontiguousarray(Wk[:, cs], np.float32),
            "wv": np.ascontiguousarray(Wv[:, cs], np.float32),
            "wo": np.ascontiguousarray(Wo[cs, :], np.float32),
            "cost": cost, "sint": sint, "rot": rot, "ones": ones,
            "masks": masks,
        })
    return in_maps


def kernel(x, Wq, Wk, Wv, Wo):
    from concourse import bass_utils

    nc = get_module()
    in_maps = make_in_maps(x, Wq, Wk, Wv, Wo)
    res = bass_utils.run_bass_kernel_spmd(nc, in_maps, core_ids=list(range(8)))
    outs = [r["out"] for r in res.results]
    out = np.empty((4, L, D), np.float32)
    for b in range(4):
        out[b] = outs[2 * b] + outs[2 * b + 1]
    return out
